# revision 1
# baseline (speedup 1.0000x reference)
"""Trainium2 Bass kernel for a dense transformer block (B=2, T=2048, C=1024, H=16).

Sharding (v2, tensor-parallel attention + ReduceScatter):
  core c -> batch b = c//4, head-group g = c%4 (heads 4g..4g+3),
  own token rows r0 = 512*g of its batch.

Per core:
  LN1 over the full sequence (token-major) -> PE-transpose -> xnT
  (feature-major).  qkv only for the core's 4 heads (q,k feature-major
  via pattern B; v token-major via pattern A with a ones column per head
  for the softmax denominator).  Attention is block-causal EXACT and
  static: every core processes all 2048 queries for its heads, so the
  causal structure is identical across cores; only 4 static staircase
  masks are needed on the diagonal band.  Scores S^T = k^T.T @ q^T (keys
  on partitions), exp on ACT (scale=1/8 folded in), av accumulates the
  denominator in row 64, normalize via reciprocal + gpsimd
  partition_broadcast.  proj partial = y_heads @ Wp[head rows] computed
  token-major (pattern A), then ReduceScatter(add) over the 4-core batch
  group hands each core the reduced rows it owns.  Residual + LN2 + MLP
  (row-sharded, fc/fc2 full) finish locally.

LayerNorm affines are folded into the following matmul weights on the
host (exact).  Matmuls run as float32r (full PE rate at N>=256);
attention q/k/v/P are bf16.
"""

from contextlib import ExitStack

import ml_dtypes
import numpy as np

import concourse.bass as bass
import concourse.tile as tile
import concourse.bacc as bacc
import concourse.mybir as mybir
from concourse.bass_utils import run_bass_kernel_spmd
from concourse.masks import make_identity

F32 = mybir.dt.float32
F32R = mybir.dt.float32r
BF16 = mybir.dt.bfloat16
ALU = mybir.AluOpType
ACTF = mybir.ActivationFunctionType

B, T, C = 2, 2048, 1024
H, DH = 16, 64
FF = 4096
EPS = 1e-5
NCORES = 8
ROWS = 512            # token rows owned per core (MLP phase)
HG = 4                # heads per core
HGF = HG * DH         # 256 head-group features
NTT = T // 128        # 16 token tiles
NOT = ROWS // 128     # 4 own token tiles
NCP = C // 128        # 8 feature partition-tiles of C
NFP = FF // 128       # 32 feature partition-tiles of FF
NQC = T // 512        # 4 query chunks
VSTRIDE = DH + 1      # v stored with a ones column per head


def r(ap, pat, **kw):
    return ap.rearrange(pat, **kw)


def build_program():
    nc = bacc.Bacc("TRN2", target_bir_lowering=False, debug=False,
                   num_devices=NCORES)

    def din(name, shape, dtype=F32):
        return nc.dram_tensor(name, list(shape), dtype, kind="ExternalInput")

    xb = din("xb", (T, C))
    xo_d = din("xo", (ROWS, C))
    cmask = din("cmask", (128, 4 * 512), BF16)
    w_qkv = din("w_qkv", (C, 3 * HGF), BF16)      # q|k|v for this head group
    w_pr = din("w_pr", (HGF, C), F32R)            # proj rows for this group
    w_fc = din("w_fc", (C, FF), F32R)
    w_fc2 = din("w_fc2", (FF, C), F32R)
    b_qk_col = din("b_qk_col", (128, 4))          # q0 q1 k0 k1 bias columns
    b_v_bc = din("b_v_bc", (128, HGF))
    b_proj_bc = din("b_proj_bc", (128, C))
    b_fc_col = din("b_fc_col", (128, 32))
    b_fc2_col = din("b_fc2_col", (128, 8))
    out = nc.dram_tensor("out", [ROWS, C], F32, kind="ExternalOutput")

    with tile.TileContext(nc) as tc, ExitStack() as ctx:
        # ---- constants ----
        cpool = ctx.enter_context(tc.tile_pool(name="const", bufs=1))
        ident = cpool.tile([128, 128], F32, tag="ident")
        make_identity(nc, ident[:])
        bqk = cpool.tile([128, 4], F32, tag="bqk")
        bvbc = cpool.tile([128, HGF], F32, tag="bvbc")
        bprbc = cpool.tile([128, C], F32, tag="bprbc")
        bfc = cpool.tile([128, 32], F32, tag="bfc")
        bfc2 = cpool.tile([128, 8], F32, tag="bfc2")
        mtile = cpool.tile([128, 4 * 512], BF16, tag="mtile")
        epsc = cpool.tile([128, 1], F32, tag="epsc")
        nc.gpsimd.memset(epsc[:], EPS)

        def load_consts():
            # deferred so these DMAs queue behind the critical first x tiles
            nc.sync.dma_start(bqk[:], b_qk_col.ap())
            nc.sync.dma_start(bvbc[:], b_v_bc.ap())
            nc.sync.dma_start(bprbc[:], b_proj_bc.ap())
            nc.sync.dma_start(bfc[:], b_fc_col.ap())
            nc.sync.dma_start(bfc2[:], b_fc2_col.ap())
            nc.sync.dma_start(mtile[:], cmask.ap())

        def layernorm_apply(spool, xt, xn_out, on_act=False):
            """xn_out = (xt - mean) * rsqrt(var + eps), rowwise over 1024."""
            st = spool.tile([128, 12], F32, tag="st")
            nc.vector.bn_stats(st[:, 0:6], xt[:, 0:512])
            nc.vector.bn_stats(st[:, 6:12], xt[:, 512:1024])
            ag = spool.tile([128, 2], F32, tag="ag")
            nc.vector.bn_aggr(ag[:], r(st, "p (c s) -> p c s", s=6))
            sd = spool.tile([128, 1], F32, tag="sd")
            nc.scalar.activation(sd[:], ag[:, 1:2], ACTF.Sqrt, bias=epsc[:],
                                 scale=1.0)
            rc = spool.tile([128, 1], F32, tag="rc")
            nc.vector.reciprocal(rc[:], sd[:])
            if on_act:
                # xn = x * r + (-mu * r), as one ACT pass (DVE stays free)
                nmr = spool.tile([128, 1], F32, tag="nmr")
                nc.vector.tensor_scalar(nmr[:], ag[:, 0:1], rc[:], -1.0,
                                        op0=ALU.mult, op1=ALU.mult)
                nc.scalar.activation(xn_out, xt, ACTF.Identity,
                                     bias=nmr[:], scale=rc[:])
            else:
                nc.vector.tensor_scalar(xn_out, xt, ag[:, 0:1], rc[:],
                                        op0=ALU.subtract, op1=ALU.mult)

        # yT: attention output, feature-major [2 x (2 heads)][128, T]
        yTpool = ctx.enter_context(tc.tile_pool(name="yT", bufs=1))
        yT = [yTpool.tile([128, T], F32R, tag=f"yT{i}", name=f"yT{i}")
              for i in range(2)]
        # DRAM bounce buffers for the collective
        drpool = ctx.enter_context(tc.tile_pool(name="dram", bufs=1,
                                                space="DRAM"))
        pp_d = drpool.tile([T, C], BF16, tag="pp_d", name="pp_d")
        rs_d = drpool.tile([ROWS, C], BF16, tag="rs_d", name="rs_d")

        with ExitStack() as qctx:
            kvq = qctx.enter_context(tc.tile_pool(name="kvq", bufs=1))
            qTb = [kvq.tile([128, T], BF16, tag=f"qT{i}", name=f"qT{i}")
                   for i in range(2)]
            kTb = [kvq.tile([128, T], BF16, tag=f"kT{i}", name=f"kT{i}")
                   for i in range(2)]
            vb = [kvq.tile([128, HG * VSTRIDE], BF16, tag=f"v{i}", name=f"v{i}")
                  for i in range(NTT)]

            # ============= Phase A: LN1 + transpose to feature-major =======
            with tc.tile_pool(name="phAxnT", bufs=1) as xnTpool:
                xnT = [xnTpool.tile([128, T], BF16, tag=f"xnT{i}",
                                    name=f"xnT{i}") for i in range(NCP)]
                with tc.tile_pool(name="phA", bufs=4) as apool, \
                     tc.tile_pool(name="phAs", bufs=6) as aspool, \
                     tc.tile_pool(name="phAps", bufs=8, space="PSUM") as atps:
                    identb = apool.tile([128, 128], BF16, tag="identb",
                                        bufs=1)
                    nc.vector.tensor_copy(identb[:], ident[:])
                    for tt in range(NTT):
                        xt = apool.tile([128, C], F32, tag="x")
                        nc.sync.dma_start(xt[:],
                                          xb.ap()[tt * 128:(tt + 1) * 128, :])
                        if tt == 1:
                            load_consts()
                        xn = apool.tile([128, C], BF16, tag="xn")
                        layernorm_apply(aspool, xt[:], xn[:], on_act=True)
                        for pt in range(NCP):
                            tp = atps.tile([128, 128], BF16, tag="tp",
                                           name=f"tp_{tt}_{pt}")
                            nc.tensor.transpose(
                                tp[:], xn[:, pt * 128:(pt + 1) * 128],
                                identb[:])
                            if pt < 2:
                                nc.scalar.copy(
                                    xnT[pt][:, tt * 128:(tt + 1) * 128], tp[:])
                            else:
                                nc.vector.tensor_copy(
                                    xnT[pt][:, tt * 128:(tt + 1) * 128], tp[:])

                # ============= Phase B: qkv for this head group ============
                with tc.tile_pool(name="wqkv", bufs=1) as wpool, \
                     tc.tile_pool(name="qkps", bufs=3, space="PSUM") as qkps:
                    wq = [wpool.tile([128, 3 * HGF], BF16, tag=f"wq{i}",
                                     name=f"wq{i}") for i in range(NCP)]
                    for kt in range(NCP):
                        nc.sync.dma_start(
                            wq[kt][:], w_qkv.ap()[kt * 128:(kt + 1) * 128, :])
                    # q,k feature-major (pattern B): M-tiles q0 q1 k0 k1
                    for m in range(4):
                        dstl = qTb if m < 2 else kTb
                        dst = dstl[m % 2]
                        for tcix in range(NQC):
                            ps = qkps.tile([128, 512], F32, tag="qk")
                            for kt in range(NCP):
                                nc.tensor.matmul(
                                    ps[:],
                                    wq[kt][:, m * 128:(m + 1) * 128],
                                    xnT[kt][:, tcix * 512:(tcix + 1) * 512],
                                    start=(kt == 0), stop=(kt == NCP - 1))
                            nc.vector.tensor_scalar(
                                dst[:, tcix * 512:(tcix + 1) * 512], ps[:],
                                bqk[:, m:m + 1], None, op0=ALU.add)
                    # v token-major (pattern A) with ones column
                    for tt in range(NTT):
                        nc.gpsimd.memset(
                            r(vb[tt], "p (h m) -> p h m",
                              m=VSTRIDE)[:, :, DH:DH + 1], 1.0)
                        ps = qkps.tile([128, HGF], F32, tag="vp")
                        for kt in range(NCP):
                            nc.tensor.matmul(
                                ps[:],
                                xnT[kt][:, tt * 128:(tt + 1) * 128],
                                wq[kt][:, 2 * HGF:3 * HGF],
                                start=(kt == 0), stop=(kt == NCP - 1))
                        dst = r(vb[tt], "p (h m) -> p h m",
                                m=VSTRIDE)[:, :, 0:DH]
                        nc.vector.tensor_tensor(
                            dst, r(ps[:], "p (h m) -> p h m", m=DH),
                            r(bvbc[:], "p (h m) -> p h m", m=DH), op=ALU.add)

            # ============= Phase C: attention (exact block-causal) =========
            with tc.tile_pool(name="att", bufs=4) as atpool, \
                 tc.tile_pool(name="attsm", bufs=3) as smpool, \
                 tc.tile_pool(name="scps", bufs=2, space="PSUM") as scps, \
                 tc.tile_pool(name="avps", bufs=1, space="PSUM") as avps:
                for pt in range(2):
                    for qc in range(NQC):
                        nkt = 4 * (qc + 1)
                        avs = [avps.tile([128, 512], F32,
                                         tag=f"av{qc % 2}{s}",
                                         name=f"av_{pt}_{qc}_{s}")
                               for s in range(2)]
                        for kp in range(nkt // 2):
                            for sub in range(2):
                                h = 2 * pt + sub
                                hb = 64 * sub
                                sc = scps.tile([128, 1024], F32, tag="sc")
                                for j in range(2):
                                    kt = 2 * kp + j
                                    nc.tensor.matmul(
                                        sc[:, j * 512:(j + 1) * 512],
                                        kTb[pt][hb:hb + 64,
                                                kt * 128:(kt + 1) * 128],
                                        qTb[pt][hb:hb + 64,
                                                qc * 512:(qc + 1) * 512],
                                        start=True, stop=True)
                                et = atpool.tile([128, 1024], BF16, tag="e")
                                nc.scalar.activation(et[:], sc[:], ACTF.Exp,
                                                     scale=0.125)
                                for j in range(2):
                                    kt = 2 * kp + j
                                    band = kt - 4 * qc
                                    if band >= 0:
                                        pm = atpool.tile([128, 512], BF16,
                                                         tag="p")
                                        nc.vector.tensor_tensor(
                                            pm[:],
                                            et[:, j * 512:(j + 1) * 512],
                                            mtile[:,
                                                  band * 512:(band + 1) * 512],
                                            op=ALU.mult)
                                        rhs_av = pm[:]
                                    else:
                                        rhs_av = et[:, j * 512:(j + 1) * 512]
                                    nc.tensor.matmul(
                                        avs[sub][0:VSTRIDE, :],
                                        vb[kt][:,
                                               h * VSTRIDE:(h + 1) * VSTRIDE],
                                        rhs_av,
                                        start=(kt == 0), stop=(kt == nkt - 1),
                                        skip_group_check=True)
                        for sub in range(2):
                            hb = 64 * sub
                            rr = smpool.tile([1, 512], F32, tag="rr")
                            nc.vector.reciprocal(rr[:],
                                                 avs[sub][DH:DH + 1, :])
                            bc = smpool.tile([64, 512], F32, tag="bc")
                            nc.gpsimd.partition_broadcast(bc[:], rr[:])
                            nc.vector.tensor_tensor(
                                yT[pt][hb:hb + 64, qc * 512:(qc + 1) * 512],
                                avs[sub][0:DH, :], bc[:], op=ALU.mult)

        # ============= Phase D: proj partial (token-major) + RS ============
        with tc.tile_pool(name="phDw", bufs=1) as dwpool, \
             tc.tile_pool(name="phDe", bufs=3) as depool, \
             tc.tile_pool(name="pps", bufs=3, space="PSUM") as pps:
            wp = [dwpool.tile([128, C], F32R, tag=f"wp{i}", name=f"wp{i}")
                  for i in range(2)]
            for i in range(2):
                nc.sync.dma_start(wp[i][:], w_pr.ap()[i * 128:(i + 1) * 128, :])
            for tt in range(NTT):
                pe = depool.tile([128, C], BF16, tag="pe")
                for cc in range(2):
                    ps = pps.tile([128, 512], F32, tag="pj")
                    for i in range(2):
                        nc.tensor.matmul(
                            ps[:], yT[i][:, tt * 128:(tt + 1) * 128],
                            wp[i][:, cc * 512:(cc + 1) * 512],
                            start=(i == 0), stop=(i == 1))
                    nc.vector.tensor_copy(pe[:, cc * 512:(cc + 1) * 512], ps[:])
                nc.sync.dma_start(pp_d[tt * 128:(tt + 1) * 128, :], pe[:])
            nc.gpsimd.collective_compute(
                "ReduceScatter", ALU.add,
                replica_groups=[[0, 1, 2, 3], [4, 5, 6, 7]],
                ins=[pp_d.opt()], outs=[rs_d.opt()])

        # ============= Phase E: residual + LN2 =============================
        dxpool = ctx.enter_context(tc.tile_pool(name="phDx", bufs=1))
        x2 = [dxpool.tile([128, C], F32, tag=f"x2{i}", name=f"x2{i}")
              for i in range(NOT)]
        xn2T = [dxpool.tile([128, ROWS], F32R, tag=f"xn2T{i}", name=f"xn2T{i}")
                for i in range(NCP)]
        with tc.tile_pool(name="phE", bufs=3) as epool, \
             tc.tile_pool(name="phEs", bufs=4) as espool, \
             tc.tile_pool(name="tps", bufs=4, space="PSUM") as tps:
            for j in range(NOT):
                rs_sb = epool.tile([128, C], BF16, tag="rs")
                nc.sync.dma_start(rs_sb[:], rs_d[j * 128:(j + 1) * 128, :])
                xot = epool.tile([128, C], F32, tag="xot")
                nc.sync.dma_start(xot[:], xo_d.ap()[j * 128:(j + 1) * 128, :])
                xr = epool.tile([128, C], F32, tag="xr")
                nc.vector.tensor_tensor(xr[:], rs_sb[:], bprbc[:], op=ALU.add)
                nc.vector.tensor_tensor(x2[j][:], xr[:], xot[:], op=ALU.add)
                xn2 = epool.tile([128, C], F32, tag="xn2")
                layernorm_apply(espool, x2[j][:], xn2[:], on_act=True)
                for pt in range(NCP):
                    tp = tps.tile([128, 128], F32, tag="tp")
                    nc.tensor.transpose(tp[:], xn2[:, pt * 128:(pt + 1) * 128],
                                        ident[:])
                    nc.vector.tensor_copy(xn2T[pt][:, j * 128:(j + 1) * 128],
                                          tp[:])

        # ============= Phase F: MLP + out ==================================
        with tc.tile_pool(name="phF", bufs=2) as fpool, \
             tc.tile_pool(name="phFh", bufs=1) as hpool, \
             tc.tile_pool(name="fps", bufs=2, space="PSUM") as fps, \
             tc.tile_pool(name="tps2", bufs=4, space="PSUM") as tps2:
            hgT = [hpool.tile([128, ROWS], F32R, tag=f"hg{i}", name=f"hg{i}")
                   for i in range(NFP)]
            outsb = [hpool.tile([128, C], F32, tag=f"os{i}", name=f"os{i}")
                     for i in range(NOT)]
            for m in range(NFP):
                wt = fpool.tile([128, C], F32R, tag="wf")
                nc.sync.dma_start(
                    r(wt[:], "p (k c) -> p k c", c=128),
                    r(w_fc.ap()[:, m * 128:(m + 1) * 128],
                      "(k p) c -> p k c", p=128))
                ps = fps.tile([128, 512], F32, tag="fc")
                for kt in range(NCP):
                    nc.tensor.matmul(
                        ps[:], wt[:, kt * 128:(kt + 1) * 128], xn2T[kt][:],
                        start=(kt == 0), stop=(kt == NCP - 1))
                nc.scalar.activation(hgT[m][:], ps[:], ACTF.Gelu,
                                     bias=bfc[:, m:m + 1], scale=1.0)
            for m2 in range(8):
                wt = fpool.tile([128, FF], F32R, tag="wf2", bufs=2)
                nc.sync.dma_start(
                    r(wt[:], "p (k c) -> p k c", c=128),
                    r(w_fc2.ap()[:, m2 * 128:(m2 + 1) * 128],
                      "(k p) c -> p k c", p=128))
                ps = fps.tile([128, 512], F32, tag="fc")
                for kt2 in range(NFP):
                    nc.tensor.matmul(
                        ps[:], wt[:, kt2 * 128:(kt2 + 1) * 128], hgT[kt2][:],
                        start=(kt2 == 0), stop=(kt2 == NFP - 1))
                y2 = fpool.tile([128, 512], F32, tag="y2")
                nc.vector.tensor_scalar(y2[:], ps[:], bfc2[:, m2:m2 + 1], None,
                                        op0=ALU.add)
                for j in range(NOT):
                    tp = tps2.tile([128, 128], F32, tag="tp")
                    nc.tensor.transpose(tp[:], y2[:, j * 128:(j + 1) * 128],
                                        ident[:])
                    nc.vector.tensor_tensor(
                        outsb[j][:, m2 * 128:(m2 + 1) * 128], tp[:],
                        x2[j][:, m2 * 128:(m2 + 1) * 128], op=ALU.add)
            for j in range(NOT):
                nc.sync.dma_start(out.ap()[j * 128:(j + 1) * 128, :],
                                  outsb[j][:])

    nc.compile()
    return nc


_NC_CACHE = None


def _get_program():
    global _NC_CACHE
    if _NC_CACHE is None:
        _NC_CACHE = build_program()
    return _NC_CACHE


def _prepare_in_maps(x, ln1_g, ln1_b, w_attn, b_attn, w_proj, b_proj,
                     ln2_g, ln2_b, w_fc, b_fc, w_fc2, b_fc2):
    x = np.asarray(x, np.float32)
    ln1_g = np.asarray(ln1_g, np.float32); ln1_b = np.asarray(ln1_b, np.float32)
    w_attn = np.asarray(w_attn, np.float32); b_attn = np.asarray(b_attn, np.float32)
    w_proj = np.asarray(w_proj, np.float32); b_proj = np.asarray(b_proj, np.float32)
    ln2_g = np.asarray(ln2_g, np.float32); ln2_b = np.asarray(ln2_b, np.float32)
    w_fc = np.asarray(w_fc, np.float32); b_fc = np.asarray(b_fc, np.float32)
    w_fc2 = np.asarray(w_fc2, np.float32); b_fc2 = np.asarray(b_fc2, np.float32)

    # Fold LayerNorm affine params into the following matmuls (exact).
    w_attn_f = ln1_g[:, None] * w_attn
    b_attn_f = b_attn + ln1_b @ w_attn
    w_fc_f = ln2_g[:, None] * w_fc
    b_fc_f = b_fc + ln2_b @ w_fc

    # 4 static staircase masks for the diagonal band:
    # mask_i[p, j] = 1 iff j >= 128*i + p (query col j attends band-tile row p)
    jj = np.arange(512)[None, :]
    pp = np.arange(128)[:, None]
    cmask = np.concatenate(
        [(jj >= 128 * i + pp) for i in range(4)],
        axis=1).astype(ml_dtypes.bfloat16)

    shared = {
        "cmask": cmask,
        "w_fc": w_fc_f,
        "w_fc2": w_fc2,
        "b_proj_bc": np.ascontiguousarray(np.broadcast_to(b_proj, (128, C))),
        "b_fc_col": np.ascontiguousarray(b_fc_f.reshape(32, 128).T),
        "b_fc2_col": np.ascontiguousarray(b_fc2.reshape(8, 128).T),
    }

    in_maps = []
    for c in range(NCORES):
        bidx = c // 4
        g = c % 4
        r0 = g * ROWS
        fsl = slice(g * HGF, (g + 1) * HGF)
        w_q = w_attn_f[:, 0 * C:1 * C][:, fsl]
        w_k = w_attn_f[:, 1 * C:2 * C][:, fsl]
        w_v = w_attn_f[:, 2 * C:3 * C][:, fsl]
        b_q = b_attn_f[0 * C:1 * C][fsl]
        b_k = b_attn_f[1 * C:2 * C][fsl]
        b_v = b_attn_f[2 * C:3 * C][fsl]
        m = dict(shared)
        m["xb"] = np.ascontiguousarray(x[bidx])
        m["xo"] = np.ascontiguousarray(x[bidx][r0:r0 + ROWS])
        m["w_qkv"] = np.ascontiguousarray(
            np.concatenate([w_q, w_k, w_v], axis=1)).astype(ml_dtypes.bfloat16)
        m["w_pr"] = np.ascontiguousarray(w_proj[fsl, :])
        m["b_qk_col"] = np.ascontiguousarray(
            np.concatenate([b_q, b_k]).reshape(4, 128).T)
        m["b_v_bc"] = np.ascontiguousarray(np.broadcast_to(b_v, (128, HGF)))
        in_maps.append(m)
    return in_maps


def _gather(res):
    y = np.empty((B, T, C), np.float32)
    for c in range(NCORES):
        bidx = c // 4
        r0 = (c % 4) * ROWS
        y[bidx, r0:r0 + ROWS] = res.results[c]["out"]
    return y


def kernel(**inputs):
    in_maps = _prepare_in_maps(**inputs)
    nc = _get_program()
    res = run_bass_kernel_spmd(nc, in_maps, core_ids=list(range(NCORES)))
    return _gather(res)


def run_traced(inputs, **kw):
    """Run with NTFF tracing; returns (output, BassKernelResults)."""
    in_maps = _prepare_in_maps(**inputs)
    nc = _get_program()
    res = run_bass_kernel_spmd(nc, in_maps, core_ids=list(range(NCORES)),
                               trace=True, **kw)
    return _gather(res), res



# revision 6
# speedup vs baseline: 1.3065x; 1.3065x over previous
"""Trainium2 Bass kernel for a dense transformer block (B=2, T=2048, C=1024, H=16).

Sharding (v3, pipelined tensor-parallel attention + chunked ReduceScatter):
  core c -> batch b = c//4, head-group g = c%4 (heads 4g..4g+3).
  After the CHUNKED ReduceScatter, core g owns token rows
  {qc*512 + 128*g + [0,128) : qc in 0..3} of its batch (strided, 4 tiles).

Program structure (single pass, emission interleaved per query chunk qc):
  for qc in 0..3:
    A(qc): LN1 + PE-transpose of token tiles 4qc..4qc+3 -> xnT (feature-major
           ring buffer)
    B(qc): q,k for this chunk (feature-major) + v token tiles (token-major,
           with a ones column per head for the softmax denominator)
    C(qc): exact block-causal attention for the core's 4 heads over this
           query chunk (keys 0..4(qc+1)); exp on ACT, masks on DVE,
           normalization via reciprocal + gpsimd partition_broadcast
    D(qc): proj partial for the chunk's 4 token tiles (token-major) +
           ReduceScatter(add) of the 512-row chunk over the 4-core batch
           group -> each core receives its 128 reduced rows
  E(j), F: residual + LN2 + MLP (fc1 feature-major, fc2 token-major -- no
           output transposes) on the core's 4 owned row tiles.  The last RS
           chunk flies while fc1 runs on earlier chunks.

Everything is bf16 except PSUM accumulation, LN statistics and the final
residual path (f32).  LayerNorm affines are folded into the following
matmul weights on the host (exact).  Weights are pre-laid-out on the host
so every DMA moves >=2KB contiguous runs per partition.
"""

from contextlib import ExitStack

import ml_dtypes
import numpy as np

import concourse.bass as bass
import concourse.tile as tile
import concourse.bacc as bacc
import concourse.mybir as mybir
from concourse.bass_utils import run_bass_kernel_spmd
from concourse.masks import make_identity

F32 = mybir.dt.float32
BF16 = mybir.dt.bfloat16
ALU = mybir.AluOpType
ACTF = mybir.ActivationFunctionType

B, T, C = 2, 2048, 1024
H, DH = 16, 64
FF = 4096
EPS = 1e-5
NCORES = 8
ROWS = 512            # token rows owned per core (MLP phase)
HG = 4                # heads per core
HGF = HG * DH         # 256 head-group features
NCP = C // 128        # 8 feature partition-tiles of C
NFP = FF // 128       # 32 feature partition-tiles of FF
NQC = T // 512        # 4 query chunks
VSTRIDE = DH + 1      # v stored with a ones column per head


def r(ap, pat, **kw):
    return ap.rearrange(pat, **kw)


def build_program():
    nc = bacc.Bacc("TRN2", target_bir_lowering=False, debug=False,
                   num_devices=NCORES)

    def din(name, shape, dtype=F32):
        return nc.dram_tensor(name, list(shape), dtype, kind="ExternalInput")

    xb = din("xb", (T, C), BF16)
    xo_d = din("xo", (ROWS, C))
    cmask = din("cmask", (128, 4 * 512), BF16)
    w_qkv = din("w_qkv", (128, NCP, 3 * HGF), BF16)   # [p, kt, q|k|v feats]
    w_pr = din("w_pr", (2, 128, C), BF16)             # proj rows, 2 p-tiles
    w_fc = din("w_fc", (128, NFP, NCP, 128), BF16)    # [p, m, kt, c]
    w_fc2 = din("w_fc2", (128, 4, NFP, 256), BF16)    # [p, cc, k2, c]
    b_qk_col = din("b_qk_col", (128, 4))              # q0 q1 k0 k1 bias cols
    b_v_bc = din("b_v_bc", (128, HGF))
    b_proj_bc = din("b_proj_bc", (128, C), BF16)
    b_fc_col = din("b_fc_col", (128, 32))
    b_fc2_bc = din("b_fc2_bc", (128, C), BF16)
    out = nc.dram_tensor("out", [ROWS, C], F32, kind="ExternalOutput")

    with tile.TileContext(nc) as tc, ExitStack() as ctx:
        # ---- constants (whole-program lifetime) ----
        cpool = ctx.enter_context(tc.tile_pool(name="const", bufs=1))
        ident = cpool.tile([128, 128], F32, tag="ident")
        make_identity(nc, ident[:])
        identb = cpool.tile([128, 128], BF16, tag="identb")
        nc.vector.tensor_copy(identb[:], ident[:])
        bqk = cpool.tile([128, 4], F32, tag="bqk")
        bvbc = cpool.tile([128, HGF], F32, tag="bvbc")
        bprbc = cpool.tile([128, C], BF16, tag="bprbc")
        bfc = cpool.tile([128, 32], F32, tag="bfc")
        bfc2bc = cpool.tile([128, C], BF16, tag="bfc2bc")
        mtile = cpool.tile([128, 4 * 512], BF16, tag="mtile")
        epsc = cpool.tile([128, 1], F32, tag="epsc")
        nc.gpsimd.memset(epsc[:], EPS)

        def load_consts():
            # deferred so these DMAs queue behind the critical first x tiles
            nc.sync.dma_start(bqk[:], b_qk_col.ap())
            nc.sync.dma_start(bvbc[:], b_v_bc.ap())
            nc.sync.dma_start(bprbc[:], b_proj_bc.ap())
            nc.sync.dma_start(bfc[:], b_fc_col.ap())
            nc.sync.dma_start(bfc2bc[:], b_fc2_bc.ap())
            nc.sync.dma_start(mtile[:], cmask.ap())

        def layernorm_apply(spool, xt, xn_out):
            """xn_out = (xt - mean) * rsqrt(var + eps), rowwise over 1024."""
            st = spool.tile([128, 12], F32, tag="st")
            nc.vector.bn_stats(st[:, 0:6], xt[:, 0:512])
            nc.vector.bn_stats(st[:, 6:12], xt[:, 512:1024])
            ag = spool.tile([128, 2], F32, tag="ag")
            nc.vector.bn_aggr(ag[:], r(st, "p (c s) -> p c s", s=6))
            sd = spool.tile([128, 1], F32, tag="sd")
            nc.scalar.activation(sd[:], ag[:, 1:2], ACTF.Sqrt, bias=epsc[:],
                                 scale=1.0)
            rc = spool.tile([128, 1], F32, tag="rc")
            nc.vector.reciprocal(rc[:], sd[:])
            # xn = x * r + (-mu * r), as one ACT pass
            nmr = spool.tile([128, 1], F32, tag="nmr")
            nc.vector.tensor_scalar(nmr[:], ag[:, 0:1], rc[:], -1.0,
                                    op0=ALU.mult, op1=ALU.mult)
            nc.scalar.activation(xn_out, xt, ACTF.Identity,
                                 bias=nmr[:], scale=rc[:])

        # DRAM bounce buffers for the chunked collective
        drpool = ctx.enter_context(tc.tile_pool(name="dram", bufs=1,
                                                space="DRAM"))
        pp_d = drpool.tile([T, C], BF16, tag="pp_d", name="pp_d")
        rs_d = drpool.tile([ROWS, C], BF16, tag="rs_d", name="rs_d")

        bidx = 0  # placeholder; replica groups are static below

        # =================== attention super-phase =====================
        with ExitStack() as actx:
            # persistent within attention
            kvp = actx.enter_context(tc.tile_pool(name="kv", bufs=1))
            kTb = [kvp.tile([128, T], BF16, tag=f"kT{i}", name=f"kT{i}")
                   for i in range(2)]
            vb = [kvp.tile([128, 2, HG * VSTRIDE], BF16, tag=f"v{i}",
                           name=f"v{i}") for i in range(T // 256)]
            wq = kvp.tile([128, NCP, 3 * HGF], BF16, tag="wq", name="wq")
            wp = [kvp.tile([128, C], BF16, tag=f"wp{i}", name=f"wp{i}")
                  for i in range(2)]
            # ring pools
            xnTp = actx.enter_context(tc.tile_pool(name="xnT", bufs=1))
            qTp = actx.enter_context(tc.tile_pool(name="qT", bufs=1))
            yTp = actx.enter_context(tc.tile_pool(name="yT", bufs=1))
            apool = actx.enter_context(tc.tile_pool(name="phA", bufs=3))
            aspool = actx.enter_context(tc.tile_pool(name="phAs", bufs=6))
            smpool = actx.enter_context(tc.tile_pool(name="sm", bufs=3))
            atpool = actx.enter_context(tc.tile_pool(name="att", bufs=4))
            depool = actx.enter_context(tc.tile_pool(name="phDe", bufs=2))
            # PSUM: atps 1 + mm 2 + sc 3 + av 2 banks = 8 banks
            atps = actx.enter_context(
                tc.tile_pool(name="atps", bufs=1, space="PSUM"))
            mmps = actx.enter_context(
                tc.tile_pool(name="mmps", bufs=2, space="PSUM"))
            scps = actx.enter_context(
                tc.tile_pool(name="scps", bufs=3, space="PSUM"))
            avps = actx.enter_context(
                tc.tile_pool(name="avps", bufs=2, space="PSUM"))

            # ones columns of v (written once, persistent)
            for i in range(T // 256):
                nc.gpsimd.memset(
                    r(vb[i], "p s (h m) -> p s h m",
                      m=VSTRIDE)[:, :, :, DH:DH + 1], 1.0)
            nc.sync.dma_start(wq[:], w_qkv.ap())
            for i in range(2):
                nc.sync.dma_start(wp[i][:], w_pr.ap()[i])

            for qc in range(NQC):
                nkt = 4 * (qc + 1)
                # ---------- A(qc): LN1 + transpose to feature-major -------
                xnTq = xnTp.tile([128, NCP, 512], BF16, tag="xnT", bufs=2,
                                 name=f"xnT{qc}")
                for tloc in range(4):
                    tt = 4 * qc + tloc
                    xt = apool.tile([128, C], BF16, tag="x")
                    nc.sync.dma_start(xt[:],
                                      xb.ap()[tt * 128:(tt + 1) * 128, :])
                    if tt == 1:
                        load_consts()
                    xn = apool.tile([128, C], BF16, tag="xn")
                    layernorm_apply(aspool, xt[:], xn[:])
                    for half in range(2):
                        tp = atps.tile([128, 512], BF16, tag="tp",
                                       name=f"tp_{tt}_{half}")
                        for pq in range(4):
                            pt = 4 * half + pq
                            nc.tensor.transpose(
                                tp[:, pq * 128:(pq + 1) * 128],
                                xn[:, pt * 128:(pt + 1) * 128],
                                identb[:])
                        nc.vector.tensor_copy(
                            xnTq[:, 4 * half:4 * half + 4,
                                 tloc * 128:(tloc + 1) * 128],
                            r(tp[:], "p (k c) -> p k c", c=128))

                # ---------- B(qc): q,k (feature-major) + v (token-major) --
                qTb = [qTp.tile([128, 512], BF16, tag=f"qT{i}", bufs=2,
                                name=f"qT{i}_{qc}") for i in range(2)]
                for m in range(4):
                    # m: 0,1 -> q head-pairs; 2,3 -> k head-pairs
                    ps = mmps.tile([128, 512], F32, tag="mm")
                    for kt in range(NCP):
                        nc.tensor.matmul(
                            ps[:], wq[:, kt, m * 128:(m + 1) * 128],
                            xnTq[:, kt, :],
                            start=(kt == 0), stop=(kt == NCP - 1))
                    if m < 2:
                        dst = qTb[m][:]
                    else:
                        dst = kTb[m - 2][:, qc * 512:(qc + 1) * 512]
                    nc.vector.tensor_scalar(dst, ps[:], bqk[:, m:m + 1], None,
                                            op0=ALU.add)
                for tloc in range(4):
                    tt = 4 * qc + tloc
                    ps = mmps.tile([128, 512], F32, tag="mm")
                    for kt in range(NCP):
                        nc.tensor.matmul(
                            ps[:, 0:HGF],
                            xnTq[:, kt, tloc * 128:(tloc + 1) * 128],
                            wq[:, kt, 2 * HGF:3 * HGF],
                            start=(kt == 0), stop=(kt == NCP - 1))
                    dst = r(vb[tt // 2][:, tt % 2, :], "p (h m) -> p h m",
                            m=VSTRIDE)[:, :, 0:DH]
                    nc.vector.tensor_tensor(
                        dst, r(ps[:, 0:HGF], "p (h m) -> p h m", m=DH),
                        r(bvbc[:], "p (h m) -> p h m", m=DH), op=ALU.add)

                # ---------- C(qc): block-causal attention ------------------
                yTq = [yTp.tile([128, 512], BF16, tag=f"yT{i}", bufs=2,
                                name=f"yT{i}_{qc}") for i in range(2)]
                for pt in range(2):
                    for sub in range(2):
                        h = 2 * pt + sub
                        hb = 64 * sub
                        avs = avps.tile([128, 512], F32, tag="av",
                                        name=f"av_{qc}_{h}")
                        for kt in range(nkt):
                            sc = scps.tile([128, 512], F32, tag="sc")
                            nc.tensor.matmul(
                                sc[:],
                                kTb[pt][hb:hb + 64, kt * 128:(kt + 1) * 128],
                                qTb[pt][hb:hb + 64, :],
                                start=True, stop=True)
                            et = atpool.tile([128, 512], BF16, tag="e")
                            nc.scalar.activation(et[:], sc[:], ACTF.Exp,
                                                 scale=0.125)
                            band = kt - 4 * qc
                            if band >= 0:
                                pm = atpool.tile([128, 512], BF16, tag="p")
                                nc.vector.tensor_tensor(
                                    pm[:], et[:],
                                    mtile[:, band * 512:(band + 1) * 512],
                                    op=ALU.mult)
                                rhs_av = pm[:]
                            else:
                                rhs_av = et[:]
                            nc.tensor.matmul(
                                avs[0:VSTRIDE, :],
                                vb[kt // 2][:, kt % 2,
                                            h * VSTRIDE:(h + 1) * VSTRIDE],
                                rhs_av,
                                start=(kt == 0), stop=(kt == nkt - 1),
                                skip_group_check=True)
                        rr = smpool.tile([1, 512], F32, tag="rr")
                        nc.vector.reciprocal(rr[:], avs[DH:DH + 1, :])
                        bc = smpool.tile([64, 512], F32, tag="bc")
                        nc.gpsimd.partition_broadcast(bc[:], rr[:])
                        nc.vector.tensor_tensor(
                            yTq[pt][hb:hb + 64, :], avs[0:DH, :], bc[:],
                            op=ALU.mult)

                # ---------- D(qc): proj partial + chunked ReduceScatter ----
                for tloc in range(4):
                    tt = 4 * qc + tloc
                    pe = depool.tile([128, C], BF16, tag="pe")
                    for cc in range(2):
                        ps = mmps.tile([128, 512], F32, tag="mm")
                        for i in range(2):
                            nc.tensor.matmul(
                                ps[:],
                                yTq[i][:, tloc * 128:(tloc + 1) * 128],
                                wp[i][:, cc * 512:(cc + 1) * 512],
                                start=(i == 0), stop=(i == 1))
                        nc.scalar.copy(pe[:, cc * 512:(cc + 1) * 512], ps[:])
                    nc.sync.dma_start(pp_d[tt * 128:(tt + 1) * 128, :], pe[:])
                nc.gpsimd.collective_compute(
                    "ReduceScatter", ALU.add,
                    replica_groups=[[0, 1, 2, 3], [4, 5, 6, 7]],
                    ins=[pp_d[qc * 512:(qc + 1) * 512, :]],
                    outs=[rs_d[qc * 128:(qc + 1) * 128, :]])

        # =================== MLP super-phase ===========================
        with ExitStack() as mctx:
            mpers = mctx.enter_context(tc.tile_pool(name="mpers", bufs=1))
            x2 = [mpers.tile([128, C], F32, tag=f"x2{j}", name=f"x2{j}")
                  for j in range(4)]
            xn2T = mpers.tile([128, NCP, ROWS], BF16, tag="xn2T",
                              name="xn2T")
            hgT = mpers.tile([128, NFP, ROWS], BF16, tag="hgT", name="hgT")
            epool = mctx.enter_context(tc.tile_pool(name="phE", bufs=2))
            espool = mctx.enter_context(tc.tile_pool(name="phEs", bufs=4))
            fpool = mctx.enter_context(tc.tile_pool(name="phF", bufs=2))
            opool = mctx.enter_context(tc.tile_pool(name="phO", bufs=2))
            etps = mctx.enter_context(
                tc.tile_pool(name="etps", bufs=2, space="PSUM"))
            fps = mctx.enter_context(
                tc.tile_pool(name="fps", bufs=2, space="PSUM"))
            f2ps = mctx.enter_context(
                tc.tile_pool(name="f2ps", bufs=2, space="PSUM"))

            # ---------- E(j): residual + LN2 + transpose ----------------
            for j in range(4):
                rs_sb = epool.tile([128, C], BF16, tag="rs")
                nc.sync.dma_start(rs_sb[:], rs_d[j * 128:(j + 1) * 128, :])
                xot = epool.tile([128, C], F32, tag="xot")
                nc.sync.dma_start(xot[:], xo_d.ap()[j * 128:(j + 1) * 128, :])
                xa = epool.tile([128, C], BF16, tag="xa")
                nc.vector.tensor_tensor(xa[:], rs_sb[:], bprbc[:], op=ALU.add)
                nc.vector.tensor_tensor(x2[j][:], xa[:], xot[:], op=ALU.add)
                xn2 = epool.tile([128, C], BF16, tag="xn2")
                layernorm_apply(espool, x2[j][:], xn2[:])
                for half in range(2):
                    tp = etps.tile([128, 512], BF16, tag="tp")
                    for pq in range(4):
                        pt = 4 * half + pq
                        nc.tensor.transpose(
                            tp[:, pq * 128:(pq + 1) * 128],
                            xn2[:, pt * 128:(pt + 1) * 128],
                            identb[:])
                    nc.vector.tensor_copy(
                        xn2T[:, 4 * half:4 * half + 4,
                             j * 128:(j + 1) * 128],
                        r(tp[:], "p (k c) -> p k c", c=128))

            # ---------- F: fc1 (feature-major) -------------------------
            for mg in range(8):     # groups of 4 m-tiles
                wf = fpool.tile([128, 4, NCP, 128], BF16, tag="wf")
                nc.sync.dma_start(wf[:], w_fc.ap()[:, 4 * mg:4 * mg + 4])
                for mloc in range(4):
                    m = 4 * mg + mloc
                    pss = [fps.tile([128, 256], F32, tag=f"fc{hh}",
                                    name=f"fc_{m}_{hh}")
                           for hh in range(2)]
                    for kt in range(NCP):
                        for hh in range(2):
                            nc.tensor.matmul(
                                pss[hh][:], wf[:, mloc, kt, :],
                                xn2T[:, kt, hh * 256:(hh + 1) * 256],
                                start=(kt == 0), stop=(kt == NCP - 1))
                    for hh in range(2):
                        nc.scalar.activation(
                            hgT[:, m, hh * 256:(hh + 1) * 256], pss[hh][:],
                            ACTF.Gelu, bias=bfc[:, m:m + 1], scale=1.0)

            # ---------- F: fc2 (token-major, no transposes) -------------
            outsb = [opool.tile([128, C], F32, tag=f"os{j}", bufs=1,
                                name=f"os{j}") for j in range(4)]
            for cc in range(4):
                wf2 = fpool.tile([128, NFP, 256], BF16, tag="wf2")
                nc.sync.dma_start(wf2[:], w_fc2.ap()[:, cc])
                for j in range(4):
                    ps = f2ps.tile([128, 256], F32, tag="f2")
                    for k2 in range(NFP):
                        nc.tensor.matmul(
                            ps[:], hgT[:, k2, j * 128:(j + 1) * 128],
                            wf2[:, k2, :],
                            start=(k2 == 0), stop=(k2 == NFP - 1))
                    ya = opool.tile([128, 256], BF16, tag="ya")
                    nc.vector.tensor_tensor(
                        ya[:], ps[:], bfc2bc[:, cc * 256:(cc + 1) * 256],
                        op=ALU.add)
                    nc.vector.tensor_tensor(
                        outsb[j][:, cc * 256:(cc + 1) * 256], ya[:],
                        x2[j][:, cc * 256:(cc + 1) * 256], op=ALU.add)
            for j in range(4):
                nc.sync.dma_start(out.ap()[j * 128:(j + 1) * 128, :],
                                  outsb[j][:])

    nc.compile()
    return nc


_NC_CACHE = None


def _get_program():
    global _NC_CACHE
    if _NC_CACHE is None:
        _NC_CACHE = build_program()
    return _NC_CACHE


def _prepare_in_maps(x, ln1_g, ln1_b, w_attn, b_attn, w_proj, b_proj,
                     ln2_g, ln2_b, w_fc, b_fc, w_fc2, b_fc2):
    bf = ml_dtypes.bfloat16
    x = np.asarray(x, np.float32)
    ln1_g = np.asarray(ln1_g, np.float32); ln1_b = np.asarray(ln1_b, np.float32)
    w_attn = np.asarray(w_attn, np.float32); b_attn = np.asarray(b_attn, np.float32)
    w_proj = np.asarray(w_proj, np.float32); b_proj = np.asarray(b_proj, np.float32)
    ln2_g = np.asarray(ln2_g, np.float32); ln2_b = np.asarray(ln2_b, np.float32)
    w_fc = np.asarray(w_fc, np.float32); b_fc = np.asarray(b_fc, np.float32)
    w_fc2 = np.asarray(w_fc2, np.float32); b_fc2 = np.asarray(b_fc2, np.float32)

    # Fold LayerNorm affine params into the following matmuls (exact).
    w_attn_f = ln1_g[:, None] * w_attn
    b_attn_f = b_attn + ln1_b @ w_attn
    w_fc_f = ln2_g[:, None] * w_fc
    b_fc_f = b_fc + ln2_b @ w_fc

    # 4 static staircase masks for the diagonal band:
    # mask_i[p, j] = 1 iff j >= 128*i + p (query col j attends band-tile row p)
    jj = np.arange(512)[None, :]
    pp = np.arange(128)[:, None]
    cmask = np.concatenate(
        [(jj >= 128 * i + pp) for i in range(4)], axis=1).astype(bf)

    # weight pre-layouts for contiguous DMA runs
    # w_fc_f [C, FF]: [k(8),p(128)] x [m(32),c(128)] -> [p, m, k, c]
    wfc_p = np.ascontiguousarray(
        w_fc_f.reshape(NCP, 128, NFP, 128).transpose(1, 2, 0, 3)).astype(bf)
    # w_fc2 [FF, C]: [k2(32),p(128)] x [cc(4),c(256)] -> [p, cc, k2, c]
    wfc2_p = np.ascontiguousarray(
        w_fc2.reshape(NFP, 128, 4, 256).transpose(1, 2, 0, 3)).astype(bf)

    shared = {
        "cmask": cmask,
        "w_fc": wfc_p,
        "w_fc2": wfc2_p,
        "b_proj_bc": np.ascontiguousarray(
            np.broadcast_to(b_proj, (128, C))).astype(bf),
        "b_fc_col": np.ascontiguousarray(b_fc_f.reshape(32, 128).T),
        "b_fc2_bc": np.ascontiguousarray(
            np.broadcast_to(b_fc2, (128, C))).astype(bf),
    }

    in_maps = []
    for c in range(NCORES):
        bidx = c // 4
        g = c % 4
        fsl = slice(g * HGF, (g + 1) * HGF)
        w_q = w_attn_f[:, 0 * C:1 * C][:, fsl]
        w_k = w_attn_f[:, 1 * C:2 * C][:, fsl]
        w_v = w_attn_f[:, 2 * C:3 * C][:, fsl]
        b_q = b_attn_f[0 * C:1 * C][fsl]
        b_k = b_attn_f[1 * C:2 * C][fsl]
        b_v = b_attn_f[2 * C:3 * C][fsl]
        m = dict(shared)
        m["xb"] = np.ascontiguousarray(x[bidx]).astype(bf)
        # owned rows: 4 tiles of 128 rows at stride 512 (chunked RS layout)
        rows = x[bidx].reshape(4, 4, 128, C)[:, g]   # [qc, 128, C]
        m["xo"] = np.ascontiguousarray(rows.reshape(ROWS, C))
        wqkv = np.concatenate([w_q, w_k, w_v], axis=1)       # [C, 768]
        m["w_qkv"] = np.ascontiguousarray(
            wqkv.reshape(NCP, 128, 3 * HGF).transpose(1, 0, 2)).astype(bf)
        m["w_pr"] = np.ascontiguousarray(
            w_proj[fsl, :].reshape(2, 128, C)).astype(bf)
        m["b_qk_col"] = np.ascontiguousarray(
            np.concatenate([b_q, b_k]).reshape(4, 128).T)
        m["b_v_bc"] = np.ascontiguousarray(np.broadcast_to(b_v, (128, HGF)))
        in_maps.append(m)
    return in_maps


def _gather(res):
    y = np.empty((B, T, C), np.float32)
    for c in range(NCORES):
        bidx = c // 4
        g = c % 4
        o = res.results[c]["out"].reshape(4, 128, C)
        for qc in range(4):
            r0 = qc * 512 + g * 128
            y[bidx, r0:r0 + 128] = o[qc]
    return y


def kernel(**inputs):
    in_maps = _prepare_in_maps(**inputs)
    nc = _get_program()
    res = run_bass_kernel_spmd(nc, in_maps, core_ids=list(range(NCORES)))
    return _gather(res)


def run_traced(inputs, **kw):
    """Run with NTFF tracing; returns (output, BassKernelResults)."""
    in_maps = _prepare_in_maps(**inputs)
    nc = _get_program()
    res = run_bass_kernel_spmd(nc, in_maps, core_ids=list(range(NCORES)),
                               trace=True, **kw)
    return _gather(res), res
